# revision 68
# baseline (speedup 1.0000x reference)
"""Bass/Tile kernel for BertUnpadSelfAttention on 8 TRN2 cores.

Problem shapes: B=4, S=1024, L=512 valid tokens/seq, H=12, D=64, DIM=768.
Sharding: core c handles batch b=c//2, heads h0=6*(c%2) .. h0+5.

Host sends ebias = exp(bias) (bf16, layout [h, 128, valid|pad chunks]).
Per-core device program (bf16 matmuls, f32 PSUM):
  qkT = wqkT.T @ xT (+bqk)         (768 feats x 512 tokens; q pre-scaled 1/8)
  v   = xT.T @ wvT (+bv)           packed [128,4,6,65], col 0 = ones
  per head j (PV pipelined one head behind QK):
    psc  = sum_c z1.T @ ebias_pad[c]      (z1 col 0 = ones -> psc[0] = pad den)
    ST   = kT_j.T-contract qT_j           (4 chunks of 128 valid k)
    e    = exp(ST)                        (ACT, PSUM->SBUF bf16)
    p    = e * ebias_valid                (DVE, bf16)
    psc += sum_c v_aug[c].T @ p[c]        (psc[0] += valid den; rows 1-64 ctx)
    rcp  = 1/psc[0]                       (DVE on PSUM partition 0)
    bc   = broadcast(rcp)                 (GpSimd partition_broadcast)
    out_j = psc[1:65] * bc                (DVE, PSUM x SBUF)
"""
import sys

sys.path.insert(0, "/opt/trn_rl_repo")

import numpy as np

import concourse.bacc as bacc
import concourse.mybir as mybir
from concourse.tile import TileContext

F32 = mybir.dt.float32
F32R = mybir.dt.float32r
BF16 = mybir.dt.bfloat16
import os as _os
import ml_dtypes as _mld
MM_DT = BF16
MM_NP = _mld.bfloat16
P = 128
B, S, L = 4, 1024, 512
H, D = 12, 64
DIM = H * D
HPC = 6          # heads per core
T = 512          # tokens per core (= L, batch resident on 2 cores)
QKF = 2 * HPC * D   # 768 q+k output features per core
VF = HPC * D        # 384 v output features per core
KC_IN = DIM // P    # 6 contraction chunks for the projection
NKC = L // P        # 4 valid-key chunks of 128
SCALE = 1.0 / 8.0
WARM_MMS = int(_os.environ.get("ATTN_WARM", "11"))


def mm(nc, out, lhsT, rhs, start, stop):
    nc.tensor.matmul(out, lhsT, rhs, start=start, stop=stop)


def build_kernel(skip_qkv_bias=False):
    nc = bacc.Bacc("TRN2", target_bir_lowering=False, debug=False, num_devices=8)

    xw = nc.dram_tensor("xw", [DIM, T + QKF + VF], MM_DT, kind="ExternalInput")
    bqk = nc.dram_tensor("bqk", [1, QKF], MM_DT, kind="ExternalInput")
    bv = nc.dram_tensor("bv", [1, VF], MM_DT, kind="ExternalInput")
    ebias = nc.dram_tensor("ebias", [HPC, P, 2 * NKC * T], MM_DT,
                           kind="ExternalInput")
    ones1 = nc.dram_tensor("ones1", [1, T], MM_DT, kind="ExternalInput")
    # row 0 = softmax denominator, rows 1-64 = unnormalized context
    out = nc.dram_tensor("out", [HPC, D + 1, T], BF16, kind="ExternalOutput")

    with TileContext(nc) as tc:
        with (
            tc.tile_pool(name="const", bufs=1) as cpool,
            tc.tile_pool(name="qkv", bufs=1) as qkvpool,
            tc.tile_pool(name="eb", bufs=HPC) as ebpool,
            tc.tile_pool(name="hexp", bufs=3) as hepool,
            tc.tile_pool(name="hp", bufs=HPC) as hppool,
            tc.tile_pool(name="hout", bufs=2) as hopool,
            tc.tile_pool(name="ps", bufs=5, space="PSUM") as pspool,
            tc.tile_pool(name="psc", bufs=3, space="PSUM") as pscpool,
        ):
            # ---- big input DMAs first: they gate everything ----
            xT_sb = []
            wqk_sb = []
            wv_sb = []
            for kc in range(KC_IN):
                xw_t = cpool.tile([P, T + QKF + VF], MM_DT, tag=f"xw{kc}")
                nc.sync.dma_start(out=xw_t[:], in_=xw[kc * P:(kc + 1) * P, :])
                xT_sb.append(xw_t[:, 0:T])
                wqk_sb.append(xw_t[:, T:T + QKF])
                wv_sb.append(xw_t[:, T + QKF:])
            eb_sb = []
            for j in range(HPC):
                eb_t = ebpool.tile([P, 2 * NKC * T], MM_DT, tag="eb")
                nc.sync.dma_start(out=eb_t[:], in_=ebias[j])
                eb_sb.append(eb_t)
            if not skip_qkv_bias:
                bqk_sb = cpool.tile([1, QKF], MM_DT, tag="bqk")
                nc.sync.dma_start(out=bqk_sb[:], in_=bqk[:])
                bv_sb = cpool.tile([1, VF], MM_DT, tag="bv")
                nc.sync.dma_start(out=bv_sb[:], in_=bv[:])
                ones_sb = cpool.tile([1, T], MM_DT, tag="ones")
                nc.sync.dma_start(out=ones_sb[:], in_=ones1[:])

            # ---- constants built on-chip (no DMA dependency) ----
            # z1: column 63 = ones -> pad matmuls accumulate the pad
            # denominator into psc partition 63 (adjacent to the context
            # rows 64-127 so ctx+den ship as one DMA)
            z1_sb = cpool.tile([P, P], MM_DT, tag="z1")
            nc.gpsimd.memset(z1_sb[:], 0.0)
            nc.gpsimd.memset(z1_sb[:, D - 1:D], 1.0)
            warm_a = cpool.tile([P, T], MM_DT, tag="warm_a")
            nc.gpsimd.memset(warm_a[:], 0.0)
            warm_w = cpool.tile([P, D], MM_DT, tag="warm_w")
            nc.gpsimd.memset(warm_w[:], 0.0)
            # v packed [128, NKC, HPC, 128]; element 0 of the last dim is a
            # ones column (accumulates the valid denominator into psc
            # partition 0); v occupies elements 64-127 so the context rows
            # land on the quadrant-aligned psc partitions 64-127
            vall = qkvpool.tile([P, NKC, HPC, P], MM_DT, tag="vall")
            nc.gpsimd.memset(vall[:], 0.0)
            nc.gpsimd.memset(vall[:, :, :, D - 1:D], 1.0)

            # ---- PE warm-up: p-state ramp while input DMAs land ----
            # narrow (256-col) matmuls: enough to keep the PE clocked up
            # without charging the HAM power integrator
            for wi in range(WARM_MMS):
                pw = pspool.tile([P, T], F32, tag="ps")
                mm(nc, pw[0:D, 0:T // 2], warm_w[:], warm_a[:, 0:T // 2],
                   start=True, stop=True)

            # ---- QKV projection (one PSUM tile per output chunk) ----
            # qkT[f, t] = sum_i wqkT[i, f] * xT[i, t] (+ bqk[f]); chunk
            # pairs (mc, mc+3) so head 2*mc's q and k land together
            qkT_sb = {}

            def issue_qk_proj(mcg):
                for mc in (mcg, mcg + 3):
                    ps = pspool.tile([P, T], F32, tag="ps")
                    for kc in range(KC_IN):
                        mm(nc, ps[:], wqk_sb[kc][:, mc * P:(mc + 1) * P],
                           xT_sb[kc], start=(kc == 0),
                           stop=(skip_qkv_bias and kc == KC_IN - 1))
                    if not skip_qkv_bias:
                        mm(nc, ps[:], bqk_sb[:, mc * P:(mc + 1) * P],
                           ones_sb[:], start=False, stop=True)
                    qt = qkvpool.tile([P, T], MM_DT, tag=f"qkT{mc}")
                    nc.vector.tensor_copy(qt[:], ps[:])
                    qkT_sb[mc] = qt

            def issue_v_proj(tcg):
                for hi in range(2):
                    tch = 2 * tcg + hi
                    ps = pspool.tile([P, T], F32, tag="ps")
                    for kc in range(KC_IN):
                        mm(nc, ps[:, 0:VF], xT_sb[kc][:, tch * P:(tch + 1) * P],
                           wv_sb[kc], start=(kc == 0),
                           stop=(skip_qkv_bias and kc == KC_IN - 1))
                    if not skip_qkv_bias:
                        mm(nc, ps[:, 0:VF], ones_sb[:, tch * P:(tch + 1) * P],
                           bv_sb[:], start=False, stop=True)
                    nc.vector.tensor_copy(
                        vall[:, tch, :, D:2 * D],
                        ps[:, 0:VF].rearrange("p (j d) -> p j d", j=HPC),
                    )

            # ---- attention ----
            psc_t = [None] * HPC
            p_t = [None] * HPC

            ebp_t = [None] * HPC

            def issue_front(j):
                """pad pre-sum + QK + exp + p-mult for head j."""
                eb = eb_sb[j]
                # pre-sum pad chunk pairs on the DVE: two z1 matmuls
                # (issued in issue_back) instead of four
                ebp = hppool.tile([P, 2, T], MM_DT, tag="ebp")
                nc.vector.tensor_add(ebp[:, 0, :],
                                     eb[:, NKC * T:(NKC + 1) * T],
                                     eb[:, (NKC + 1) * T:(NKC + 2) * T])
                nc.vector.tensor_add(ebp[:, 1, :],
                                     eb[:, (NKC + 2) * T:(NKC + 3) * T],
                                     eb[:, (NKC + 3) * T:(NKC + 4) * T])
                ebp_t[j] = ebp
                qT_h = qkT_sb[j // 2][(j % 2) * D:(j % 2) * D + D, :]
                kT_h = qkT_sb[3 + j // 2][(j % 2) * D:(j % 2) * D + D, :]
                exp_v = hepool.tile([P, NKC * T], MM_DT, tag="exp_v")
                p = hppool.tile([P, NKC * T], MM_DT, tag="p")
                for kc in range(NKC):
                    ps = pspool.tile([P, T], F32, tag="ps")
                    mm(nc, ps[:], kT_h[:, kc * P:(kc + 1) * P], qT_h,
                       start=True, stop=True)
                    nc.scalar.activation(
                        exp_v[:, kc * T:(kc + 1) * T], ps[:],
                        mybir.ActivationFunctionType.Exp,
                    )
                    nc.vector.tensor_mul(
                        p[:, kc * T:(kc + 1) * T],
                        exp_v[:, kc * T:(kc + 1) * T],
                        eb[:, kc * T:(kc + 1) * T],
                    )
                p_t[j] = p

            def issue_back(j):
                """PV + pad denominator + output for head j (the final
                division by the denominator happens on the host during
                unsharding)."""
                psc = pscpool.tile([P, T], F32, tag="psc")
                p = p_t[j]
                eb = eb_sb[j]
                ebp = ebp_t[j]
                for kc in range(NKC - 1):
                    mm(nc, psc[:], vall[:, kc, j, :],
                       p[:, kc * T:(kc + 1) * T],
                       start=(kc == 0), stop=False)
                mm(nc, psc[:], z1_sb[:], ebp[:, 0, :], start=False, stop=False)
                mm(nc, psc[:], z1_sb[:], ebp[:, 1, :], start=False, stop=False)
                mm(nc, psc[:], vall[:, NKC - 1, j, :],
                   p[:, (NKC - 1) * T:NKC * T], start=False, stop=True)
                outh = hopool.tile([P, T], BF16, tag="outh")
                nc.vector.tensor_copy(outh[:], psc[:])
                nc.gpsimd.dma_start(out=out[j], in_=outh[D - 1:P, :])

            for mcg in range(3):
                issue_qk_proj(mcg)
            issue_front(0)
            issue_front(1)
            issue_v_proj(0)
            issue_front(2)
            issue_v_proj(1)
            for j in range(3, HPC):
                issue_back(j - 3)
                issue_front(j)
            issue_back(HPC - 3)
            issue_back(HPC - 2)
            issue_back(HPC - 1)

    nc.compile()
    return nc


# ---------------- host-side sharding ----------------

def make_core_inputs(hidden_states, Wqkv_w, Wqkv_b, bias, core):
    b, half = core // 2, core % 2
    h0 = HPC * half
    xT = np.ascontiguousarray(hidden_states[b * T:(b + 1) * T, :].T)
    wq = Wqkv_w[h0 * D:(h0 + HPC) * D, :] * np.float32(SCALE)
    wk = Wqkv_w[DIM + h0 * D:DIM + (h0 + HPC) * D, :]
    wv = Wqkv_w[2 * DIM + h0 * D:2 * DIM + (h0 + HPC) * D, :]
    wqkT = np.ascontiguousarray(np.concatenate([wq, wk], axis=0).T)
    wvT = np.ascontiguousarray(wv.T)
    bq = Wqkv_b[h0 * D:(h0 + HPC) * D] * np.float32(SCALE)
    bk = Wqkv_b[DIM + h0 * D:DIM + (h0 + HPC) * D]
    bv_ = Wqkv_b[2 * DIM + h0 * D:2 * DIM + (h0 + HPC) * D]
    bqk = np.ascontiguousarray(np.concatenate([bq, bk])[None, :])
    bv = np.ascontiguousarray(bv_[None, :])
    bt = bias[b, h0:h0 + HPC, :T, :].transpose(0, 2, 1)   # (h, k, q)
    ebias = np.ascontiguousarray(
        np.exp(bt.astype(np.float32)).reshape(HPC, 2, NKC, P, T)
        .transpose(0, 3, 1, 2, 4).reshape(HPC, P, 2 * NKC * T)
    )
    return dict(
        xw=np.concatenate([xT, wqkT, wvT], axis=1).astype(MM_NP),
        bqk=bqk.astype(MM_NP),
        bv=bv.astype(MM_NP),
        ebias=ebias.astype(MM_NP),
        ones1=np.ones((1, T), MM_NP),
    )


def assemble_output(core_outs):
    full = np.empty((B * T, DIM), np.float32)
    for core, outd in enumerate(core_outs):
        b, half = core // 2, core % 2
        h0 = HPC * half
        outd = outd.astype(np.float32)
        arr = outd[:, 1:, :] / outd[:, 0:1, :]
        full[b * T:(b + 1) * T, h0 * D:(h0 + HPC) * D] = (
            arr.transpose(2, 0, 1).reshape(T, HPC * D)
        )
    return full


def core_reference(ci):
    """numpy reference of the per-core shard computation -> (HPC, D, T)."""
    # unpack ebias [h, p, 2, c, q] -> [h, k, q] with k = v*512 + c*128 + p
    eb = (ci["ebias"].astype(np.float32)
          .reshape(HPC, P, 2, NKC, T).transpose(0, 2, 3, 1, 4)
          .reshape(HPC, S, T))
    xw_ = ci["xw"].astype(np.float32)
    xT_ = xw_[:, 0:T]
    wqkT_ = xw_[:, T:T + QKF]
    qkT = wqkT_.T @ xT_ + ci["bqk"].astype(np.float32).T       # (768, 512)
    v = xT_.T @ xw_[:, T + QKF:] + ci["bv"].astype(np.float32)
    outs = []
    for j in range(HPC):
        qT = qkT[j * D:(j + 1) * D, :]                # (64, 512)
        kT = qkT[VF + j * D:VF + (j + 1) * D, :]      # (64, 512)
        ep_v = np.exp(kT.T @ qT) * eb[j, :L, :]       # (512k, 512q)
        ep_p = eb[j, L:, :]                           # (512k_pad, 512q)
        vh = v[:, j * D:(j + 1) * D]                  # (512, 64)
        ctx = vh.T @ ep_v                             # (64, 512)
        den = ep_v.sum(axis=0) + ep_p.sum(axis=0)     # (512,)
        outs.append(ctx / den[None, :])
    return np.stack(outs)


# ---------------- public entry point ----------------

_NC_CACHE = {}


def _get_nc(skip_qkv_bias):
    key = skip_qkv_bias
    if key not in _NC_CACHE:
        _NC_CACHE[key] = build_kernel(skip_qkv_bias=skip_qkv_bias)
    return _NC_CACHE[key]


def _canonical(hidden_states, Wqkv_w, Wqkv_b, bias, indices, attn_mask,
               cu_seqlens, max_seqlen_in_batch):
    if hidden_states.shape != (B * T, DIM) or Wqkv_w.shape != (3 * DIM, DIM):
        return False
    if bias.shape != (B, H, S, S) or indices.shape != (B * T,):
        return False
    if int(max_seqlen_in_batch) != S or attn_mask.shape != (B, S):
        return False
    want = (np.arange(B)[:, None] * S + np.arange(T)[None, :]).reshape(-1)
    return bool((indices.astype(np.int64) == want).all())


def _reference_fallback(hidden_states, Wqkv_w, Wqkv_b, bias, indices,
                        attn_mask, cu_seqlens, max_seqlen_in_batch):
    b = attn_mask.shape[0]
    s = int(max_seqlen_in_batch)
    h = bias.shape[1]
    d = Wqkv_w.shape[1] // h
    qkv = hidden_states.astype(np.float32) @ Wqkv_w.astype(np.float32).T
    qkv = qkv + Wqkv_b.astype(np.float32)
    padded = np.zeros((b * s, qkv.shape[-1]), np.float32)
    padded[indices.astype(np.int64)] = qkv
    qkv = padded.reshape(b, s, 3, h, d)
    q, k, v = qkv[:, :, 0], qkv[:, :, 1], qkv[:, :, 2]
    scale = 1.0 / float(np.sqrt(d))
    scores = np.einsum("bqhd,bkhd->bhqk", q, k) * scale
    scores = scores + bias.astype(np.float32)
    scores -= scores.max(axis=-1, keepdims=True)
    probs = np.exp(scores)
    probs /= probs.sum(axis=-1, keepdims=True)
    ctx = np.einsum("bhqk,bkhd->bqhd", probs, v)
    return ctx.reshape(b * s, h * d)[indices.astype(np.int64)].astype(np.float32)


def kernel(hidden_states, Wqkv_w, Wqkv_b, bias, indices, attn_mask,
           cu_seqlens, max_seqlen_in_batch):
    hidden_states = np.asarray(hidden_states)
    Wqkv_w = np.asarray(Wqkv_w)
    Wqkv_b = np.asarray(Wqkv_b)
    bias = np.asarray(bias)
    indices = np.asarray(indices)
    attn_mask = np.asarray(attn_mask)

    if not _canonical(hidden_states, Wqkv_w, Wqkv_b, bias, indices,
                      attn_mask, cu_seqlens, max_seqlen_in_batch):
        return _reference_fallback(hidden_states, Wqkv_w, Wqkv_b, bias,
                                   indices, attn_mask, cu_seqlens,
                                   max_seqlen_in_batch)

    from concourse.bass_utils import run_bass_kernel_spmd

    skip_bias = bool((Wqkv_b == 0).all())
    nc = _get_nc(skip_bias)
    in_maps = [
        make_core_inputs(hidden_states, Wqkv_w, Wqkv_b, bias, core)
        for core in range(8)
    ]
    out = None
    for _ in range(4):
        res = run_bass_kernel_spmd(nc, in_maps, list(range(8)))
        out = assemble_output([res.results[c]["out"] for c in range(8)])
        # softmax-averaged values are bounded ~O(1); garbage from a rare
        # device-side fault is astronomically larger - rerun if detected
        if np.isfinite(out).all() and np.abs(out).max() < 10.0:
            break
    return out


# revision 69
# speedup vs baseline: 1.0185x; 1.0185x over previous
"""Bass/Tile kernel for BertUnpadSelfAttention on 8 TRN2 cores.

Problem shapes: B=4, S=1024, L=512 valid tokens/seq, H=12, D=64, DIM=768.
Sharding: core c handles batch b=c//2, heads h0=6*(c%2) .. h0+5.

Host sends ebias = exp(bias) (bf16, layout [h, 128, valid|pad chunks]).
Per-core device program (bf16 matmuls, f32 PSUM):
  qkT = wqkT.T @ xT (+bqk)         (768 feats x 512 tokens; q pre-scaled 1/8)
  v   = xT.T @ wvT (+bv)           packed [128,4,6,65], col 0 = ones
  per head j (PV pipelined one head behind QK):
    psc  = sum_c z1.T @ ebias_pad[c]      (z1 col 0 = ones -> psc[0] = pad den)
    ST   = kT_j.T-contract qT_j           (4 chunks of 128 valid k)
    e    = exp(ST)                        (ACT, PSUM->SBUF bf16)
    p    = e * ebias_valid                (DVE, bf16)
    psc += sum_c v_aug[c].T @ p[c]        (psc[0] += valid den; rows 1-64 ctx)
    rcp  = 1/psc[0]                       (DVE on PSUM partition 0)
    bc   = broadcast(rcp)                 (GpSimd partition_broadcast)
    out_j = psc[1:65] * bc                (DVE, PSUM x SBUF)
"""
import sys

sys.path.insert(0, "/opt/trn_rl_repo")

import numpy as np

import concourse.bacc as bacc
import concourse.mybir as mybir
from concourse.tile import TileContext

F32 = mybir.dt.float32
F32R = mybir.dt.float32r
BF16 = mybir.dt.bfloat16
import os as _os
import ml_dtypes as _mld
MM_DT = BF16
MM_NP = _mld.bfloat16
P = 128
B, S, L = 4, 1024, 512
H, D = 12, 64
DIM = H * D
HPC = 6          # heads per core
T = 512          # tokens per core (= L, batch resident on 2 cores)
QKF = 2 * HPC * D   # 768 q+k output features per core
VF = HPC * D        # 384 v output features per core
KC_IN = DIM // P    # 6 contraction chunks for the projection
NKC = L // P        # 4 valid-key chunks of 128
SCALE = 1.0 / 8.0
WARM_MMS = int(_os.environ.get("ATTN_WARM", "11"))


def mm(nc, out, lhsT, rhs, start, stop):
    nc.tensor.matmul(out, lhsT, rhs, start=start, stop=stop)


def build_kernel(skip_qkv_bias=False):
    nc = bacc.Bacc("TRN2", target_bir_lowering=False, debug=False, num_devices=8)

    xw = nc.dram_tensor("xw", [DIM, T + QKF + VF], MM_DT, kind="ExternalInput")
    bqk = nc.dram_tensor("bqk", [1, QKF], MM_DT, kind="ExternalInput")
    bv = nc.dram_tensor("bv", [1, VF], MM_DT, kind="ExternalInput")
    ebias = nc.dram_tensor("ebias", [HPC, P, 2 * NKC * T], MM_DT,
                           kind="ExternalInput")
    ones1 = nc.dram_tensor("ones1", [1, T], MM_DT, kind="ExternalInput")
    # row 0 = softmax denominator, rows 1-64 = unnormalized context
    out = nc.dram_tensor("out", [HPC, D + 1, T], BF16, kind="ExternalOutput")

    with TileContext(nc) as tc:
        with (
            tc.tile_pool(name="const", bufs=1) as cpool,
            tc.tile_pool(name="qkv", bufs=1) as qkvpool,
            tc.tile_pool(name="eb", bufs=HPC) as ebpool,
            tc.tile_pool(name="hexp", bufs=3) as hepool,
            tc.tile_pool(name="hp", bufs=HPC) as hppool,
            tc.tile_pool(name="hout", bufs=2) as hopool,
            tc.tile_pool(name="ps", bufs=5, space="PSUM") as pspool,
            tc.tile_pool(name="psc", bufs=3, space="PSUM") as pscpool,
        ):
            # ---- big input DMAs first: they gate everything ----
            xT_sb = []
            wqk_sb = []
            wv_sb = []
            for kc in range(KC_IN):
                xw_t = cpool.tile([P, T + QKF + VF], MM_DT, tag=f"xw{kc}")
                nc.sync.dma_start(out=xw_t[:], in_=xw[kc * P:(kc + 1) * P, :])
                xT_sb.append(xw_t[:, 0:T])
                wqk_sb.append(xw_t[:, T:T + QKF])
                wv_sb.append(xw_t[:, T + QKF:])
            eb_sb = []
            for j in range(HPC):
                eb_t = ebpool.tile([P, 2 * NKC * T], MM_DT, tag="eb")
                nc.sync.dma_start(out=eb_t[:], in_=ebias[j])
                eb_sb.append(eb_t)
            if not skip_qkv_bias:
                bqk_sb = cpool.tile([1, QKF], MM_DT, tag="bqk")
                nc.sync.dma_start(out=bqk_sb[:], in_=bqk[:])
                bv_sb = cpool.tile([1, VF], MM_DT, tag="bv")
                nc.sync.dma_start(out=bv_sb[:], in_=bv[:])
                ones_sb = cpool.tile([1, T], MM_DT, tag="ones")
                nc.sync.dma_start(out=ones_sb[:], in_=ones1[:])

            # ---- constants built on-chip (no DMA dependency) ----
            # z1: column 63 = ones -> pad matmuls accumulate the pad
            # denominator into psc partition 63 (adjacent to the context
            # rows 64-127 so ctx+den ship as one DMA)
            z1_sb = cpool.tile([P, P], MM_DT, tag="z1")
            nc.gpsimd.memset(z1_sb[:], 0.0)
            nc.gpsimd.memset(z1_sb[:, D - 1:D], 1.0)
            warm_a = cpool.tile([P, T], MM_DT, tag="warm_a")
            nc.gpsimd.memset(warm_a[:], 0.0)
            warm_w = cpool.tile([P, D], MM_DT, tag="warm_w")
            nc.gpsimd.memset(warm_w[:], 0.0)
            # v packed [128, NKC, HPC, 128]; element 0 of the last dim is a
            # ones column (accumulates the valid denominator into psc
            # partition 0); v occupies elements 64-127 so the context rows
            # land on the quadrant-aligned psc partitions 64-127
            vall = qkvpool.tile([P, NKC, HPC, P], MM_DT, tag="vall")
            nc.gpsimd.memset(vall[:], 0.0)
            nc.gpsimd.memset(vall[:, :, :, D - 1:D], 1.0)

            # ---- PE warm-up: p-state ramp while input DMAs land ----
            # narrow (256-col) matmuls: enough to keep the PE clocked up
            # without charging the HAM power integrator
            for wi in range(WARM_MMS):
                pw = pspool.tile([P, T], F32, tag="ps")
                mm(nc, pw[0:D, 0:T // 2], warm_w[:], warm_a[:, 0:T // 2],
                   start=True, stop=True)

            # ---- QKV projection (one PSUM tile per output chunk) ----
            # qkT[f, t] = sum_i wqkT[i, f] * xT[i, t] (+ bqk[f]); chunk
            # pairs (mc, mc+3) so head 2*mc's q and k land together
            qkT_sb = {}

            def issue_qk_proj(mcg):
                for mc in (mcg, mcg + 3):
                    ps = pspool.tile([P, T], F32, tag="ps")
                    for kc in range(KC_IN):
                        mm(nc, ps[:], wqk_sb[kc][:, mc * P:(mc + 1) * P],
                           xT_sb[kc], start=(kc == 0),
                           stop=(skip_qkv_bias and kc == KC_IN - 1))
                    if not skip_qkv_bias:
                        mm(nc, ps[:], bqk_sb[:, mc * P:(mc + 1) * P],
                           ones_sb[:], start=False, stop=True)
                    qt = qkvpool.tile([P, T], MM_DT, tag=f"qkT{mc}")
                    nc.vector.tensor_copy(qt[:], ps[:])
                    qkT_sb[mc] = qt

            def issue_v_proj(tcg):
                for hi in range(2):
                    tch = 2 * tcg + hi
                    ps = pspool.tile([P, T], F32, tag="ps")
                    for kc in range(KC_IN):
                        mm(nc, ps[:, 0:VF], xT_sb[kc][:, tch * P:(tch + 1) * P],
                           wv_sb[kc], start=(kc == 0),
                           stop=(skip_qkv_bias and kc == KC_IN - 1))
                    if not skip_qkv_bias:
                        mm(nc, ps[:, 0:VF], ones_sb[:, tch * P:(tch + 1) * P],
                           bv_sb[:], start=False, stop=True)
                    nc.vector.tensor_copy(
                        vall[:, tch, :, D:2 * D],
                        ps[:, 0:VF].rearrange("p (j d) -> p j d", j=HPC),
                    )

            # ---- attention ----
            psc_t = [None] * HPC
            p_t = [None] * HPC

            ebp_t = [None] * HPC

            def issue_front(j):
                """pad pre-sum + QK + exp + p-mult for head j."""
                eb = eb_sb[j]
                # pre-sum pad chunk pairs on the DVE: two z1 matmuls
                # (issued in issue_back) instead of four
                ebp = hppool.tile([P, 2, T], MM_DT, tag="ebp")
                nc.vector.tensor_add(ebp[:, 0, :],
                                     eb[:, NKC * T:(NKC + 1) * T],
                                     eb[:, (NKC + 1) * T:(NKC + 2) * T])
                nc.vector.tensor_add(ebp[:, 1, :],
                                     eb[:, (NKC + 2) * T:(NKC + 3) * T],
                                     eb[:, (NKC + 3) * T:(NKC + 4) * T])
                ebp_t[j] = ebp
                qT_h = qkT_sb[j // 2][(j % 2) * D:(j % 2) * D + D, :]
                kT_h = qkT_sb[3 + j // 2][(j % 2) * D:(j % 2) * D + D, :]
                exp_v = hepool.tile([P, NKC * T], MM_DT, tag="exp_v")
                p = hppool.tile([P, NKC * T], MM_DT, tag="p")
                for kc in range(NKC):
                    ps = pspool.tile([P, T], F32, tag="ps")
                    mm(nc, ps[:], kT_h[:, kc * P:(kc + 1) * P], qT_h,
                       start=True, stop=True)
                    nc.scalar.activation(
                        exp_v[:, kc * T:(kc + 1) * T], ps[:],
                        mybir.ActivationFunctionType.Exp,
                    )
                    nc.vector.tensor_mul(
                        p[:, kc * T:(kc + 1) * T],
                        exp_v[:, kc * T:(kc + 1) * T],
                        eb[:, kc * T:(kc + 1) * T],
                    )
                p_t[j] = p

            def issue_back(j):
                """PV + pad denominator + output for head j (the final
                division by the denominator happens on the host during
                unsharding)."""
                psc = pscpool.tile([P, T], F32, tag="psc")
                p = p_t[j]
                eb = eb_sb[j]
                ebp = ebp_t[j]
                for kc in range(NKC):
                    mm(nc, psc[:], vall[:, kc, j, :],
                       p[:, kc * T:(kc + 1) * T],
                       start=(kc == 0), stop=False)
                mm(nc, psc[:], z1_sb[:], ebp[:, 0, :], start=False, stop=False)
                mm(nc, psc[:], z1_sb[:], ebp[:, 1, :], start=False, stop=True)
                outh = hopool.tile([P, T], BF16, tag="outh")
                nc.vector.tensor_copy(outh[:], psc[:])
                nc.gpsimd.dma_start(out=out[j], in_=outh[D - 1:P, :])

            for mcg in range(3):
                issue_qk_proj(mcg)
            issue_front(0)
            issue_front(1)
            issue_v_proj(0)
            issue_front(2)
            issue_v_proj(1)
            for j in range(3, HPC):
                issue_back(j - 3)
                issue_front(j)
            issue_back(HPC - 3)
            issue_back(HPC - 2)
            issue_back(HPC - 1)

    nc.compile()
    return nc


# ---------------- host-side sharding ----------------

def make_core_inputs(hidden_states, Wqkv_w, Wqkv_b, bias, core):
    b, half = core // 2, core % 2
    h0 = HPC * half
    xT = np.ascontiguousarray(hidden_states[b * T:(b + 1) * T, :].T)
    wq = Wqkv_w[h0 * D:(h0 + HPC) * D, :] * np.float32(SCALE)
    wk = Wqkv_w[DIM + h0 * D:DIM + (h0 + HPC) * D, :]
    wv = Wqkv_w[2 * DIM + h0 * D:2 * DIM + (h0 + HPC) * D, :]
    wqkT = np.ascontiguousarray(np.concatenate([wq, wk], axis=0).T)
    wvT = np.ascontiguousarray(wv.T)
    bq = Wqkv_b[h0 * D:(h0 + HPC) * D] * np.float32(SCALE)
    bk = Wqkv_b[DIM + h0 * D:DIM + (h0 + HPC) * D]
    bv_ = Wqkv_b[2 * DIM + h0 * D:2 * DIM + (h0 + HPC) * D]
    bqk = np.ascontiguousarray(np.concatenate([bq, bk])[None, :])
    bv = np.ascontiguousarray(bv_[None, :])
    bt = bias[b, h0:h0 + HPC, :T, :].transpose(0, 2, 1)   # (h, k, q)
    ebias = np.ascontiguousarray(
        np.exp(bt.astype(np.float32)).reshape(HPC, 2, NKC, P, T)
        .transpose(0, 3, 1, 2, 4).reshape(HPC, P, 2 * NKC * T)
    )
    return dict(
        xw=np.concatenate([xT, wqkT, wvT], axis=1).astype(MM_NP),
        bqk=bqk.astype(MM_NP),
        bv=bv.astype(MM_NP),
        ebias=ebias.astype(MM_NP),
        ones1=np.ones((1, T), MM_NP),
    )


def assemble_output(core_outs):
    full = np.empty((B * T, DIM), np.float32)
    for core, outd in enumerate(core_outs):
        b, half = core // 2, core % 2
        h0 = HPC * half
        outd = outd.astype(np.float32)
        arr = outd[:, 1:, :] / outd[:, 0:1, :]
        full[b * T:(b + 1) * T, h0 * D:(h0 + HPC) * D] = (
            arr.transpose(2, 0, 1).reshape(T, HPC * D)
        )
    return full


def core_reference(ci):
    """numpy reference of the per-core shard computation -> (HPC, D, T)."""
    # unpack ebias [h, p, 2, c, q] -> [h, k, q] with k = v*512 + c*128 + p
    eb = (ci["ebias"].astype(np.float32)
          .reshape(HPC, P, 2, NKC, T).transpose(0, 2, 3, 1, 4)
          .reshape(HPC, S, T))
    xw_ = ci["xw"].astype(np.float32)
    xT_ = xw_[:, 0:T]
    wqkT_ = xw_[:, T:T + QKF]
    qkT = wqkT_.T @ xT_ + ci["bqk"].astype(np.float32).T       # (768, 512)
    v = xT_.T @ xw_[:, T + QKF:] + ci["bv"].astype(np.float32)
    outs = []
    for j in range(HPC):
        qT = qkT[j * D:(j + 1) * D, :]                # (64, 512)
        kT = qkT[VF + j * D:VF + (j + 1) * D, :]      # (64, 512)
        ep_v = np.exp(kT.T @ qT) * eb[j, :L, :]       # (512k, 512q)
        ep_p = eb[j, L:, :]                           # (512k_pad, 512q)
        vh = v[:, j * D:(j + 1) * D]                  # (512, 64)
        ctx = vh.T @ ep_v                             # (64, 512)
        den = ep_v.sum(axis=0) + ep_p.sum(axis=0)     # (512,)
        outs.append(ctx / den[None, :])
    return np.stack(outs)


# ---------------- public entry point ----------------

_NC_CACHE = {}


def _get_nc(skip_qkv_bias):
    key = skip_qkv_bias
    if key not in _NC_CACHE:
        _NC_CACHE[key] = build_kernel(skip_qkv_bias=skip_qkv_bias)
    return _NC_CACHE[key]


def _canonical(hidden_states, Wqkv_w, Wqkv_b, bias, indices, attn_mask,
               cu_seqlens, max_seqlen_in_batch):
    if hidden_states.shape != (B * T, DIM) or Wqkv_w.shape != (3 * DIM, DIM):
        return False
    if bias.shape != (B, H, S, S) or indices.shape != (B * T,):
        return False
    if int(max_seqlen_in_batch) != S or attn_mask.shape != (B, S):
        return False
    want = (np.arange(B)[:, None] * S + np.arange(T)[None, :]).reshape(-1)
    return bool((indices.astype(np.int64) == want).all())


def _reference_fallback(hidden_states, Wqkv_w, Wqkv_b, bias, indices,
                        attn_mask, cu_seqlens, max_seqlen_in_batch):
    b = attn_mask.shape[0]
    s = int(max_seqlen_in_batch)
    h = bias.shape[1]
    d = Wqkv_w.shape[1] // h
    qkv = hidden_states.astype(np.float32) @ Wqkv_w.astype(np.float32).T
    qkv = qkv + Wqkv_b.astype(np.float32)
    padded = np.zeros((b * s, qkv.shape[-1]), np.float32)
    padded[indices.astype(np.int64)] = qkv
    qkv = padded.reshape(b, s, 3, h, d)
    q, k, v = qkv[:, :, 0], qkv[:, :, 1], qkv[:, :, 2]
    scale = 1.0 / float(np.sqrt(d))
    scores = np.einsum("bqhd,bkhd->bhqk", q, k) * scale
    scores = scores + bias.astype(np.float32)
    scores -= scores.max(axis=-1, keepdims=True)
    probs = np.exp(scores)
    probs /= probs.sum(axis=-1, keepdims=True)
    ctx = np.einsum("bhqk,bkhd->bqhd", probs, v)
    return ctx.reshape(b * s, h * d)[indices.astype(np.int64)].astype(np.float32)


def kernel(hidden_states, Wqkv_w, Wqkv_b, bias, indices, attn_mask,
           cu_seqlens, max_seqlen_in_batch):
    hidden_states = np.asarray(hidden_states)
    Wqkv_w = np.asarray(Wqkv_w)
    Wqkv_b = np.asarray(Wqkv_b)
    bias = np.asarray(bias)
    indices = np.asarray(indices)
    attn_mask = np.asarray(attn_mask)

    if not _canonical(hidden_states, Wqkv_w, Wqkv_b, bias, indices,
                      attn_mask, cu_seqlens, max_seqlen_in_batch):
        return _reference_fallback(hidden_states, Wqkv_w, Wqkv_b, bias,
                                   indices, attn_mask, cu_seqlens,
                                   max_seqlen_in_batch)

    from concourse.bass_utils import run_bass_kernel_spmd

    skip_bias = bool((Wqkv_b == 0).all())
    nc = _get_nc(skip_bias)
    in_maps = [
        make_core_inputs(hidden_states, Wqkv_w, Wqkv_b, bias, core)
        for core in range(8)
    ]
    out = None
    for _ in range(4):
        res = run_bass_kernel_spmd(nc, in_maps, list(range(8)))
        out = assemble_output([res.results[c]["out"] for c in range(8)])
        # softmax-averaged values are bounded ~O(1); garbage from a rare
        # device-side fault is astronomically larger - rerun if detected
        if np.isfinite(out).all() and np.abs(out).max() < 10.0:
            break
    return out
